# revision 1
# baseline (speedup 1.0000x reference)
"""LocallyConnected1d Trainium2 kernel.

Problem: out[b, oc, w] = sum_{ic,k} xp[b, ic, w+k] * W[w, oc, ic, k] + bias[oc, w]
  x: (32, 64, 2048) f32, weights: (2048, 64, 64, 3) f32, bias: (64, 2048) f32
  out: (32, 64, 2048) f32.  xp = x padded by 1 on both sides of the last axis.

Sharding: output_width (2048) is split into 8 contiguous chunks of 256, one per
NeuronCore.  Weights dominate the traffic (100 MB) and are fully sharded this
way (12.6 MB/core); x is sent with a 2-column halo.

Per-core compute: for each position w the contraction over (ic, k) + bias is a
193-term dot product, done as two PSUM-accumulated fp32 matmuls:
  mm1: K=128 rows = (k=0, ic=0..63) ++ (k=1, ic=0..63),  lhsT=[128, 64oc], rhs=[128, 32b]
  mm2: K=65  rows = (k=2, ic=0..63) ++ bias row,         lhsT=[65, 64oc],  rhs=[65, 32b]
The bias is folded in as lhsT row 64 of mm2 against a constant ones row in rhs.

fp32 matmuls lower to 2 HW passes (LDW+MM each); with N=32 the MM pass costs
N*4 = 128 PE cycles, so the PE floor is ~512 cyc/position at the observed
1.2 GHz clock (~110 us/core).  DMA (21 MB/core) is packet-rate-bound, so
weights/x are fetched in fat 64-position slices (4-16 KB contiguous per
partition) while PSUM/compute runs in 16-position chunks (1 bank each).

Host-side prep (numpy, cheap vs. the 100MB HBM traffic on device):
  wa[j, w, oc] = W[ws+w, oc, j%64, j//64]        j in [0,128)   (k-major)
  wb[j, w, oc] = W[ws+w, oc, j, 2] for j<64;  wb[64, w, oc] = bias[oc, ws+w]
  x1[j, c, b]  = xp[b, j%64, ws+c + j//64]       j in [0,128)
  x2[j, c, b]  = xp[b, j, ws+c+2] for j<64;   x2[64, c, b] = 1.0
"""

import numpy as np

import concourse.bacc as bacc
import concourse.mybir as mybir
import concourse.tile as tile
from concourse.bass_utils import run_bass_kernel_spmd

B, IC, OC, KS, W = 32, 64, 64, 3, 2048
NCORES = 8
OWC = W // NCORES  # 256 positions per core
CH = 16            # compute chunk; psum tile = [64, CH*32] = one bank
DCH = 64           # DMA chunk (positions per weight/x fetch)
DT = mybir.dt.float32

_compiled_nc = None


def _build_nc():
    nc = bacc.Bacc("TRN2")

    x1_d = nc.dram_tensor("x1", [2 * IC, OWC, B], DT, kind="ExternalInput")
    x2_d = nc.dram_tensor("x2", [IC + 1, OWC, B], DT, kind="ExternalInput")
    wa_d = nc.dram_tensor("wa", [2 * IC, OWC, OC], DT, kind="ExternalInput")
    wb_d = nc.dram_tensor("wb", [IC + 1, OWC, OC], DT, kind="ExternalInput")
    out_d = nc.dram_tensor("out", [OC, OWC, B], DT, kind="ExternalOutput")

    # First DMA slice is small so the PE starts quickly; the rest are fat.
    dma_slices = [(0, CH), (CH, DCH - CH)]
    p = DCH
    while p < OWC:
        dma_slices.append((p, min(DCH, OWC - p)))
        p += DCH

    with tile.TileContext(nc) as tc:
        with (
            tc.tile_pool(name="w", bufs=2) as wpool,
            tc.tile_pool(name="x", bufs=2) as xpool,
            tc.tile_pool(name="o", bufs=3) as opool,
            tc.tile_pool(name="ps", bufs=4, space="PSUM") as pspool,
        ):
            loaded = []  # (start, len, wa, wb, x1, x2)

            def load_slice(si):
                p0, plen = dma_slices[si]
                sl = slice(p0, p0 + plen)
                wa = wpool.tile([2 * IC, plen, OC], DT, tag="wa", name=f"wa_{si}")
                wb = wpool.tile([IC + 1, plen, OC], DT, tag="wb", name=f"wb_{si}")
                x1 = xpool.tile([2 * IC, plen, B], DT, tag="x1", name=f"x1_{si}")
                x2 = xpool.tile([IC + 1, plen, B], DT, tag="x2", name=f"x2_{si}")
                # slice 0 gates the PE start: split its loads across the two
                # HWDGE queues (sync + scalar) so descriptor issue overlaps.
                eng2 = nc.scalar if si == 0 else nc.sync
                nc.sync.dma_start(out=wa[:], in_=wa_d[:, sl, :])
                nc.sync.dma_start(out=x1[:], in_=x1_d[:, sl, :])
                eng2.dma_start(out=wb[:], in_=wb_d[:, sl, :])
                eng2.dma_start(out=x2[:], in_=x2_d[:, sl, :])
                loaded.append((p0, plen, wa, wb, x1, x2))

            # Software-pipelined emission: loads for slice si+1 are emitted
            # just before slice si's compute, so the HWDGE queue never holds
            # more than ~1 slice of prefetch during the ramp and the critical
            # early slices get the DMA engines to themselves.
            load_slice(0)
            load_slice(1)
            for si in range(len(dma_slices)):
                if si >= 1 and si + 1 < len(dma_slices):
                    load_slice(si + 1)
                p0, plen, wa, wb, x1, x2 = loaded[si]
                for c0 in range(0, plen, CH):
                    cl = min(CH, plen - c0)
                    ps = pspool.tile([OC, cl, B], DT, tag="ps", name=f"ps_{p0 + c0}")
                    for w in range(cl):
                        wl = c0 + w
                        nc.tensor.matmul(
                            ps[:, w, :],
                            wa[:, wl, :],
                            x1[:, wl, :],
                            start=True,
                            stop=False,
                        )
                        nc.tensor.matmul(
                            ps[:, w, :],
                            wb[:, wl, :],
                            x2[:, wl, :],
                            start=False,
                            stop=True,
                        )
                    ob = opool.tile([OC, cl, B], DT, tag="ob", name=f"ob_{p0 + c0}")
                    nc.scalar.copy(out=ob[:], in_=ps[:])
                    nc.sync.dma_start(
                        out=out_d[:, p0 + c0 : p0 + c0 + cl, :], in_=ob[:]
                    )

    nc.compile()
    return nc


def _get_nc():
    global _compiled_nc
    if _compiled_nc is None:
        _compiled_nc = _build_nc()
    return _compiled_nc


def shard_inputs(x, weights, bias):
    x = np.ascontiguousarray(np.asarray(x, dtype=np.float32))
    weights = np.asarray(weights, dtype=np.float32)
    bias = np.asarray(bias, dtype=np.float32)

    xp = np.pad(x, ((0, 0), (0, 0), (1, 1)))
    xpT = np.ascontiguousarray(xp.transpose(1, 2, 0))  # (IC, W+2, B)
    ones = np.ones((1, OWC, B), np.float32)

    in_maps = []
    for c in range(NCORES):
        ws = c * OWC
        x1 = np.concatenate(
            [xpT[:, ws : ws + OWC, :], xpT[:, ws + 1 : ws + 1 + OWC, :]], axis=0
        )
        x2 = np.concatenate([xpT[:, ws + 2 : ws + 2 + OWC, :], ones], axis=0)
        wsl = weights[ws : ws + OWC]  # (OWC, OC, IC, KS)
        wa = np.ascontiguousarray(wsl[:, :, :, 0:2].transpose(3, 2, 0, 1)).reshape(
            2 * IC, OWC, OC
        )
        wb = np.concatenate(
            [wsl[:, :, :, 2].transpose(2, 0, 1), bias[:, ws : ws + OWC].T[None]],
            axis=0,
        )
        in_maps.append(
            {
                "x1": np.ascontiguousarray(x1),
                "x2": np.ascontiguousarray(x2),
                "wa": np.ascontiguousarray(wa),
                "wb": np.ascontiguousarray(wb),
            }
        )
    return in_maps


def run_sharded(x, weights, bias, trace=False):
    nc = _get_nc()
    in_maps = shard_inputs(x, weights, bias)
    res = run_bass_kernel_spmd(nc, in_maps, list(range(NCORES)), trace=trace)
    out = np.empty((B, OC, W), np.float32)
    for c in range(NCORES):
        out[:, :, c * OWC : (c + 1) * OWC] = res.results[c]["out"].transpose(2, 0, 1)
    return out, res


def kernel(x, weights, bias):
    out, _ = run_sharded(x, weights, bias)
    return out



# revision 2
# speedup vs baseline: 1.9928x; 1.9928x over previous
"""LocallyConnected1d Trainium2 kernel (bf16, paired-position matmuls).

Problem: out[b, oc, w] = sum_{ic,k} xp[b, ic, w+k] * W[w, oc, ic, k] + bias[oc, w]
  x: (32, 64, 2048) f32, weights: (2048, 64, 64, 3) f32, bias: (64, 2048) f32
  out: (32, 64, 2048) f32.  xp = x padded by 1 on both sides of the last axis.

Sharding: output_width (2048) split into 8 chunks of 256, one per core.

Math per core: positions are processed in PAIRS (p, p+1).  Each pair needs the
6 tap-matrices W[p,:,:,0..2], W[p+1,:,:,0..2] contracted with x columns
p..p+3.  Two bf16 matmuls with FULL 128x128 stationary operands cover all six:

  rhs column tile xpp[:, c, :] stacks x cols (c, c+1) in 128 partitions
  (xpp[j, c, b] = xp[j%64, ws + c + j//64, b]).

  MM1 (rhs = xpp[:, p]):   stationary S1 blocks [row-block x-col, col-block pos]:
     (p+0 -> p): tap0   (p+1 -> p): tap1   (p+0 -> p+1): ZERO  (p+1 -> p+1): tap0
  MM2 (rhs = xpp[:, p+2]): stationary S2:
     (p+2 -> p): tap2   (p+3 -> p): ZERO   (p+2 -> p+1): tap1  (p+3 -> p+1): tap2

  PSUM out [128, 32]: partitions 0-63 = oc of pos p, 64-127 = oc of pos p+1.

The two ZERO blocks sit at fixed offsets in every stationary, so they are
memset once per SBUF weight buffer and never shipped over DMA: weight traffic
stays at the minimal 6.3 MB/core (bf16).  fp32 matmuls cost 4 PE cycles/row
and double LDWEIGHTS; bf16 costs 1 cycle/row and gets FWL (fast weight load)
on the full-128-column stationaries, cutting TensorE time ~4-6x vs the fp32
baseline.  Bias is folded in during PSUM evacuation on the Vector engine with
a stride-0 (broadcast over batch) access pattern.  Total DMA ~9.5 MB/core vs
~21 MB fp32 baseline.  bf16 rounding keeps rel err ~3e-3, well under the 2e-2
gate.

Host-side prep (numpy): pair-major weight re-layouts fullA/fullB (128 rows) +
halfA/halfB (64 rows), xpp, bias in pair layout.  Casts to bf16 via ml_dtypes.
"""

import ml_dtypes
import numpy as np

import concourse.bacc as bacc
import concourse.mybir as mybir
import concourse.tile as tile
from concourse.bass import broadcast_tensor_aps
from concourse.bass_utils import run_bass_kernel_spmd

B, IC, OC, KS, W = 32, 64, 64, 3, 2048
NCORES = 8
OWC = W // NCORES      # 256 positions per core
NQ = OWC // 2          # 128 position-pairs per core
GRP = 16               # pairs per PSUM bank / evacuation group
NGRP = NQ // GRP       # 8 groups
DT = mybir.dt.float32
BF = mybir.dt.bfloat16
BF_NP = ml_dtypes.bfloat16

# weight slice schedule (pairs): small first so the PE starts quickly
SLICES = [(0, 4), (4, 12), (16, 16), (32, 32), (64, 32), (96, 32)]
# xpp column splits (col c feeds pairs q with 2q or 2q+2 == c)
XSPLITS = [(0, 68), (68, 64), (132, 125)]

_compiled_nc = None


def _build_nc():
    nc = bacc.Bacc("TRN2")

    fullA_d = nc.dram_tensor("fullA", [128, NQ, OC], BF, kind="ExternalInput")
    fullB_d = nc.dram_tensor("fullB", [128, NQ, OC], BF, kind="ExternalInput")
    halfA_d = nc.dram_tensor("halfA", [64, NQ, OC], BF, kind="ExternalInput")
    halfB_d = nc.dram_tensor("halfB", [64, NQ, OC], BF, kind="ExternalInput")
    xpp_d = nc.dram_tensor("xpp", [128, OWC + 1, B], BF, kind="ExternalInput")
    bias_d = nc.dram_tensor("biasq", [128, NQ, 1], DT, kind="ExternalInput")
    out_d = nc.dram_tensor("out", [128, NQ, B], BF, kind="ExternalOutput")

    with tile.TileContext(nc) as tc:
        with (
            tc.tile_pool(name="w", bufs=1) as wpool,
            tc.tile_pool(name="x", bufs=1) as xpool,
            tc.tile_pool(name="o", bufs=4) as opool,
            tc.tile_pool(name="ps", bufs=1, space="PSUM") as pspool,
        ):
            xpp = xpool.tile([128, OWC + 1, B], BF, tag="xpp", name="xpp")
            biast = xpool.tile([128, NQ, 1], DT, tag="biast", name="biast")
            wabs = [
                wpool.tile([128, 32, 256], BF, tag=f"wab{i}", name=f"wab{i}")
                for i in range(3)
            ]
            pss = [
                pspool.tile([128, GRP, B], DT, tag=f"ps{i}", name=f"ps{i}")
                for i in range(4)
            ]

            # x first (gates the first matmuls), then bias (first needed at
            # the first group evacuation) -- on the scalar HWDGE queue so the
            # weight stream owns the sync queue.
            for c0, cl in XSPLITS:
                nc.scalar.dma_start(
                    out=xpp[:, c0 : c0 + cl, :], in_=xpp_d[:, c0 : c0 + cl, :]
                )
            nc.scalar.dma_start(out=biast[:], in_=bias_d[:])

            # The two zero blocks of every stationary are position-independent:
            # memset them once per weight buffer; DMA only writes data blocks.
            for wt in wabs:
                nc.vector.memset(wt[0:64, :, 64:128], 0.0)
                nc.vector.memset(wt[64:128, :, 128:192], 0.0)

            def load_slice(si):
                q0, L = SLICES[si]
                wt = wabs[si % 3]
                sl = slice(q0, q0 + L)
                nc.sync.dma_start(out=wt[:, 0:L, 0:64], in_=fullA_d[:, sl, :])
                nc.sync.dma_start(out=wt[64:128, 0:L, 64:128], in_=halfA_d[:, sl, :])
                nc.sync.dma_start(out=wt[0:64, 0:L, 128:192], in_=halfB_d[:, sl, :])
                nc.sync.dma_start(out=wt[:, 0:L, 192:256], in_=fullB_d[:, sl, :])

            load_slice(0)
            load_slice(1)
            for si, (q0, L) in enumerate(SLICES):
                if si >= 1 and si + 1 < len(SLICES):
                    load_slice(si + 1)
                wt = wabs[si % 3]
                for i in range(L):
                    q = q0 + i
                    g = q // GRP
                    ps = pss[g % 4]
                    slot = q % GRP
                    p = 2 * q
                    nc.tensor.matmul(
                        ps[:, slot, :],
                        wt[:, i, 0:128],
                        xpp[:, p, :],
                        start=True,
                        stop=False,
                    )
                    nc.tensor.matmul(
                        ps[:, slot, :],
                        wt[:, i, 128:256],
                        xpp[:, p + 2, :],
                        start=False,
                        stop=True,
                    )
                    if slot == GRP - 1:
                        # evacuate the finished bank: out = psum + bias
                        # (bias broadcast over the 32-batch inner dim)
                        ob = opool.tile([128, GRP, B], BF, tag="ob", name=f"ob{g}")
                        ps_ap, bias_ap = broadcast_tensor_aps(
                            ps[:, :, :], biast[:, g * GRP : (g + 1) * GRP, 0:1]
                        )
                        nc.vector.scalar_tensor_tensor(
                            out=ob[:],
                            in0=ps_ap,
                            scalar=0.0,
                            in1=bias_ap,
                            op0=mybir.AluOpType.bypass,
                            op1=mybir.AluOpType.add,
                        )
                        nc.scalar.dma_start(
                            out=out_d[:, g * GRP : (g + 1) * GRP, :], in_=ob[:]
                        )

    nc.compile()
    return nc


def _get_nc():
    global _compiled_nc
    if _compiled_nc is None:
        _compiled_nc = _build_nc()
    return _compiled_nc


def shard_inputs(x, weights, bias):
    x = np.asarray(x, dtype=np.float32)
    weights = np.asarray(weights, dtype=np.float32)
    bias = np.asarray(bias, dtype=np.float32)

    xp = np.pad(x, ((0, 0), (0, 0), (1, 1)))  # (B, IC, W+2)
    xpT = np.ascontiguousarray(xp.transpose(1, 2, 0))  # (IC, W+2, B)

    in_maps = []
    for c in range(NCORES):
        ws = c * OWC
        xc = xpT[:, ws : ws + OWC + 2, :]  # (64, 258, 32)
        xpp = np.concatenate(
            [xc[:, 0 : OWC + 1, :], xc[:, 1 : OWC + 2, :]], axis=0
        )  # (128, 257, 32)

        Wc = weights[ws : ws + OWC]  # (256, OC, IC, 3)
        We = Wc[0::2].transpose(3, 2, 0, 1)  # (3, IC, NQ, OC) even positions
        Wo = Wc[1::2].transpose(3, 2, 0, 1)  # odd positions
        fullA = np.concatenate([We[0], We[1]], axis=0)  # (128, NQ, OC)
        fullB = np.concatenate([Wo[1], Wo[2]], axis=0)
        halfA = Wo[0]  # (64, NQ, OC)
        halfB = We[2]

        bc = bias[:, ws : ws + OWC]  # (64, 256)
        biasq = np.concatenate([bc[:, 0::2], bc[:, 1::2]], axis=0)  # (128, NQ)

        in_maps.append(
            {
                "fullA": np.ascontiguousarray(fullA).astype(BF_NP),
                "fullB": np.ascontiguousarray(fullB).astype(BF_NP),
                "halfA": np.ascontiguousarray(halfA).astype(BF_NP),
                "halfB": np.ascontiguousarray(halfB).astype(BF_NP),
                "xpp": np.ascontiguousarray(xpp).astype(BF_NP),
                "biasq": np.ascontiguousarray(biasq[:, :, None]),
            }
        )
    return in_maps


def unshard_output(results):
    out = np.empty((B, OC, W), np.float32)
    for c in range(NCORES):
        ws = c * OWC
        r = np.asarray(results[c]["out"], dtype=np.float32)  # (128, NQ, B)
        # r[s*64+oc, q, b] -> out[b, oc, ws + 2q + s]
        rr = r.reshape(2, OC, NQ, B).transpose(3, 1, 2, 0)  # (B, OC, NQ, 2)
        out[:, :, ws : ws + OWC] = rr.reshape(B, OC, OWC)
    return out


def run_sharded(x, weights, bias, trace=False):
    nc = _get_nc()
    in_maps = shard_inputs(x, weights, bias)
    res = run_bass_kernel_spmd(nc, in_maps, list(range(NCORES)), trace=trace)
    return unshard_output(res.results), res


def kernel(x, weights, bias):
    out, _ = run_sharded(x, weights, bias)
    return out


# revision 12
# speedup vs baseline: 2.4938x; 1.2514x over previous
"""LocallyConnected1d Trainium2 kernel (bf16, paired-position matmuls).

Problem: out[b, oc, w] = sum_{ic,k} xp[b, ic, w+k] * W[w, oc, ic, k] + bias[oc, w]
  x: (32, 64, 2048) f32, weights: (2048, 64, 64, 3) f32, bias: (64, 2048) f32
  out: (32, 64, 2048) f32.  xp = x padded by 1 on both sides of the last axis.

Sharding: output_width (2048) split into 8 chunks of 256, one per core.

Math per core: positions are processed in PAIRS (p, p+1).  Each pair needs the
6 tap-matrices W[p,:,:,0..2], W[p+1,:,:,0..2] contracted with x columns
p..p+3.  Two bf16 matmuls with FULL 128x128 stationary operands cover all six:

  rhs column tile xpp[:, c, :] stacks x cols (c, c+1) in 128 partitions
  (xpp[j, c, b] = xp[j%64, ws + c + j//64, b]).

  MM1 (rhs = xpp[:, p]):   stationary S1 blocks [row-block x-col, col-block pos]:
     (p+0 -> p): tap0   (p+1 -> p): tap1   (p+0 -> p+1): ZERO  (p+1 -> p+1): tap0
  MM2 (rhs = xpp[:, p+2]): stationary S2:
     (p+2 -> p): tap2   (p+3 -> p): ZERO   (p+2 -> p+1): tap1  (p+3 -> p+1): tap2

  PSUM out [128, 32]: partitions 0-63 = oc of pos p, 64-127 = oc of pos p+1.

The two ZERO blocks sit at fixed offsets in every stationary, so they are
memset once per SBUF weight buffer and never shipped over DMA: weight traffic
stays at the minimal 6.3 MB/core (bf16).  fp32 matmuls cost 4 PE cycles/row
and double LDWEIGHTS; bf16 costs 1 cycle/row and gets FWL (fast weight load)
on the full-128-column stationaries, cutting TensorE time ~4-6x vs the fp32
baseline.  Bias is folded in during PSUM evacuation on the Vector engine with
a stride-0 (broadcast over batch) access pattern.  Total DMA ~9.5 MB/core vs
~21 MB fp32 baseline.  bf16 rounding keeps rel err ~3e-3, well under the 2e-2
gate.

Host-side prep (numpy): pair-major weight re-layouts fullA/fullB (128 rows) +
halfA/halfB (64 rows), xpp, bias in pair layout.  Casts to bf16 via ml_dtypes.
"""

import ml_dtypes
import numpy as np

import concourse.bacc as bacc
import concourse.mybir as mybir
import concourse.tile as tile
from concourse.bass import broadcast_tensor_aps
from concourse.bass_utils import run_bass_kernel_spmd

B, IC, OC, KS, W = 32, 64, 64, 3, 2048
NCORES = 8
OWC = W // NCORES      # 256 positions per core
NQ = OWC // 2          # 128 position-pairs per core
GRP = 16               # pairs per PSUM bank / evacuation group
NGRP = NQ // GRP       # 8 groups
DT = mybir.dt.float32
BF = mybir.dt.bfloat16
BF_NP = ml_dtypes.bfloat16

# weight slice schedule (pairs): small first so the PE starts quickly
SLICES = [(0, 4), (4, 12), (16, 16), (32, 32), (64, 32), (96, 32)]
# xpp column splits (col c feeds pairs q with 2q or 2q+2 == c)
XSPLITS = [(0, 68), (68, 64), (132, 125)]

_compiled_nc = None


def _build_nc():
    nc = bacc.Bacc("TRN2")

    # Weight DRAM tensors are flat, packed slice-major in (oc, pair) order so
    # that each per-slice DMA is a single fully-contiguous range per
    # partition on BOTH sides.  SBUF weight tiles are pair-minor
    # [128, 256 cols, L]: the matmul stationary for pair i is the 1D strided
    # AP wab[:, c0:c0+128, i] (stride L), and DMA dest col-block c for all L
    # pairs is the contiguous range [c*L, (c+64)*L).
    fullA_d = nc.dram_tensor("fullA", [128, NQ * OC], BF, kind="ExternalInput")
    fullB_d = nc.dram_tensor("fullB", [128, NQ * OC], BF, kind="ExternalInput")
    halfA_d = nc.dram_tensor("halfA", [64, NQ * OC], BF, kind="ExternalInput")
    halfB_d = nc.dram_tensor("halfB", [64, NQ * OC], BF, kind="ExternalInput")
    xpp_d = nc.dram_tensor("xpp", [128, OWC + 1, B], BF, kind="ExternalInput")
    bias_d = nc.dram_tensor("biasq", [128, NQ, 1], DT, kind="ExternalInput")
    out_d = nc.dram_tensor("out", [128, NQ, B], BF, kind="ExternalOutput")

    with tile.TileContext(nc) as tc:
        with (
            tc.tile_pool(name="w", bufs=1) as wpool,
            tc.tile_pool(name="x", bufs=1) as xpool,
            tc.tile_pool(name="o", bufs=4) as opool,
            tc.tile_pool(name="ps", bufs=1, space="PSUM") as pspool,
        ):
            xpp = xpool.tile([128, OWC + 1, B], BF, tag="xpp", name="xpp")
            biast = xpool.tile([128, NQ, 1], DT, tag="biast", name="biast")
            # one persistent weight tile per slice (no buffer reuse; ~64KB
            # per partition total), pair-minor layout [128, 256, L]
            wabs = [
                wpool.tile([128, 256, L], BF, tag=f"wab{si}", name=f"wab{si}")
                for si, (q0, L) in enumerate(SLICES)
            ]
            pss = [
                pspool.tile([128, GRP, B], DT, tag=f"ps{i}", name=f"ps{i}")
                for i in range(4)
            ]

            # x first (gates the first matmuls), then bias (first needed at
            # the first group evacuation) -- on the scalar HWDGE queue so the
            # weight stream owns the sync queue.
            for c0, cl in XSPLITS:
                nc.scalar.dma_start(
                    out=xpp[:, c0 : c0 + cl, :], in_=xpp_d[:, c0 : c0 + cl, :]
                )
            nc.scalar.dma_start(out=biast[:], in_=bias_d[:])

            # The two zero blocks of every stationary are position-independent:
            # memset them once per weight buffer; DMA only writes data blocks.
            # the two zero blocks of S1/S2 are memset once per slice tile
            for wt in wabs:
                nc.vector.memset(wt[0:64, 64:128, :], 0.0)
                nc.vector.memset(wt[64:128, 128:192, :], 0.0)

            def load_slice(si):
                q0, L = SLICES[si]
                wt = wabs[si]
                fl = slice(OC * q0, OC * (q0 + L))
                nc.sync.dma_start(out=wt[:, 0:64, :], in_=fullA_d[:, fl])
                nc.sync.dma_start(out=wt[64:128, 64:128, :], in_=halfA_d[:, fl])
                nc.sync.dma_start(out=wt[0:64, 128:192, :], in_=halfB_d[:, fl])
                nc.sync.dma_start(out=wt[:, 192:256, :], in_=fullB_d[:, fl])

            for si in range(len(SLICES)):
                load_slice(si)
            for si, (q0, L) in enumerate(SLICES):
                wt = wabs[si]
                for i in range(L):
                    q = q0 + i
                    g = q // GRP
                    ps = pss[g % 4]
                    slot = q % GRP
                    p = 2 * q
                    nc.tensor.matmul(
                        ps[:, slot, :],
                        wt[:, 0:128, i],
                        xpp[:, p, :],
                        start=True,
                        stop=False,
                    )
                    nc.tensor.matmul(
                        ps[:, slot, :],
                        wt[:, 128:256, i],
                        xpp[:, p + 2, :],
                        start=False,
                        stop=True,
                    )
                    if slot == GRP - 1:
                        # evacuate the finished bank: out = psum + bias
                        # (bias broadcast over the 32-batch inner dim)
                        ob = opool.tile([128, GRP, B], BF, tag="ob", name=f"ob{g}")
                        ps_ap, bias_ap = broadcast_tensor_aps(
                            ps[:, :, :], biast[:, g * GRP : (g + 1) * GRP, 0:1]
                        )
                        nc.vector.scalar_tensor_tensor(
                            out=ob[:],
                            in0=ps_ap,
                            scalar=0.0,
                            in1=bias_ap,
                            op0=mybir.AluOpType.bypass,
                            op1=mybir.AluOpType.add,
                        )
                        nc.scalar.dma_start(
                            out=out_d[:, g * GRP : (g + 1) * GRP, :], in_=ob[:]
                        )

    nc.compile()
    return nc


def _get_nc():
    global _compiled_nc
    if _compiled_nc is None:
        _compiled_nc = _build_nc()
    return _compiled_nc


def shard_inputs(x, weights, bias):
    x = np.asarray(x, dtype=np.float32)
    weights = np.asarray(weights, dtype=np.float32)
    bias = np.asarray(bias, dtype=np.float32)

    xp = np.pad(x, ((0, 0), (0, 0), (1, 1)))  # (B, IC, W+2)
    xpT = np.ascontiguousarray(xp.transpose(1, 2, 0))  # (IC, W+2, B)

    in_maps = []
    for c in range(NCORES):
        ws = c * OWC
        xc = xpT[:, ws : ws + OWC + 2, :]  # (64, 258, 32)
        xpp = np.concatenate(
            [xc[:, 0 : OWC + 1, :], xc[:, 1 : OWC + 2, :]], axis=0
        )  # (128, 257, 32)

        Wc = weights[ws : ws + OWC]  # (256, OC, IC, 3)
        We = Wc[0::2].transpose(3, 2, 0, 1)  # (3, IC, NQ, OC) even positions
        Wo = Wc[1::2].transpose(3, 2, 0, 1)  # odd positions
        fullA = np.concatenate([We[0], We[1]], axis=0)  # (128, NQ, OC)
        fullB = np.concatenate([Wo[1], Wo[2]], axis=0)
        halfA = Wo[0]  # (64, NQ, OC)
        halfB = We[2]

        def sliced_flat(arr):
            # [P, NQ, OC] -> [P, NQ*OC], slice-major, (oc, pair)-ordered to
            # match the pair-minor SBUF tile layout
            parts = [
                arr[:, q0 : q0 + L, :].transpose(0, 2, 1).reshape(arr.shape[0], -1)
                for q0, L in SLICES
            ]
            return np.concatenate(parts, axis=1)

        bc = bias[:, ws : ws + OWC]  # (64, 256)
        biasq = np.concatenate([bc[:, 0::2], bc[:, 1::2]], axis=0)  # (128, NQ)

        in_maps.append(
            {
                "fullA": np.ascontiguousarray(sliced_flat(fullA)).astype(BF_NP),
                "fullB": np.ascontiguousarray(sliced_flat(fullB)).astype(BF_NP),
                "halfA": np.ascontiguousarray(sliced_flat(halfA)).astype(BF_NP),
                "halfB": np.ascontiguousarray(sliced_flat(halfB)).astype(BF_NP),
                "xpp": np.ascontiguousarray(xpp).astype(BF_NP),
                "biasq": np.ascontiguousarray(biasq[:, :, None]),
            }
        )
    return in_maps


def unshard_output(results):
    out = np.empty((B, OC, W), np.float32)
    for c in range(NCORES):
        ws = c * OWC
        r = np.asarray(results[c]["out"], dtype=np.float32)  # (128, NQ, B)
        # r[s*64+oc, q, b] -> out[b, oc, ws + 2q + s]
        rr = r.reshape(2, OC, NQ, B).transpose(3, 1, 2, 0)  # (B, OC, NQ, 2)
        out[:, :, ws : ws + OWC] = rr.reshape(B, OC, OWC)
    return out


def run_sharded(x, weights, bias, trace=False):
    nc = _get_nc()
    in_maps = shard_inputs(x, weights, bias)
    res = run_bass_kernel_spmd(nc, in_maps, list(range(NCORES)), trace=trace)
    return unshard_output(res.results), res


def kernel(x, weights, bias):
    out, _ = run_sharded(x, weights, bias)
    return out
